# revision 11
# baseline (speedup 1.0000x reference)
"""ACT-R recurrence kernel, v3: PWL-exp2 on DVE + Pool-scan reduce.

Layout: batch on partitions ([128 p] x [16 bc]), j innermost on SBUF
tiles; per core B=2048.  Steps processed in PAIRS (super-step k handles
rows a=2k+1, b=2k+2) with a two-color D state in PSUM:
  DD[p, 0, j, bc] = t_a - t_j   (odd snapshot)
  DD[p, 1, j, bc] = t_b - t_j   (even snapshot)
maintained by PE broadcast-matmul prefix adds + one-column inits.

Per super-step (bulk rectangle j in [0, 2k+1), uniform for both rows —
row a needs no separate fresh term):
  Ln   (ACT):  LR = f16(Ln(scale * DD[:, :, 0:2k+2, :]))      1 inst
  QK   (DVE):  QK = LR * NEG2           f16, 2x    (NEG2 = -d*1024*log2e)
  U1   (DVE):  (QK max -14724) + 15226  -> i16, 4x   \  2-point PWL exp2:
  U2   (DVE):  (QK max -14724) + 14724  -> i16, 4x   /  bitcast i16->f16
  scan (Pool): cumsum(bitcast(U1) + bitcast(U2)) f32, flat across rows+bc
  ends (Pool): P[g] = cum[(g+1)*jb] - cum[g*jb]  (zero slot at 0)
  NP   (Pool): NEG2[:, :, a:b+1] = P * K1G + K0
  fresh (DVE): NEG2[b] += K1G * pwl2(LR_b[2k+1] * NEG2[a])
Final: S = (NEG2 - K0)/K1E; out = Sigmoid((Ln(S) - tau)/s).
The exp never touches ACT; the reduce never runs at DVE 1x.
"""

import sys

for _p in ("/opt/trn_rl_repo",):
    if _p not in sys.path:
        sys.path.insert(0, _p)

import numpy as np
from contextlib import ExitStack

import concourse.hw_specs as hw_specs
import concourse.bacc as bacc_mod
from concourse import mybir

_orig_gat = hw_specs.get_activation_tables


def _patched_gat(arch):
    tabs = _orig_gat(arch)
    out = {}
    ln_t = mybir.ActivationFunctionType.Ln
    exp_t = mybir.ActivationFunctionType.Exp
    for name, funcs in tabs.items():
        f = set(funcs)
        if name != "natural_log_exp_and_others":
            f.discard(ln_t)
            f.discard(exp_t)
        out[name] = f
    return out


bacc_mod.get_activation_tables = _patched_gat

import concourse.bass as bass
import concourse.bacc as bacc
import concourse.tile as tile
from concourse.bass_utils import run_bass_kernel_spmd

S = 128
B_FULL = 16384
N_CORES = 8
B = B_FULL // N_CORES  # 2048 per core
P = 128
NC = B // P  # 16
NK = S // 2  # 64 super-steps
NV = 4 * NK  # gm values per (p, bc)

F32 = mybir.dt.float32
F16 = mybir.dt.float16
I16 = mybir.dt.int16
BF16 = mybir.dt.bfloat16
AF = mybir.ActivationFunctionType
ALU = mybir.AluOpType

LOG2E = 1.4426950408889634
GAMMA = 0.6175430020692696
BIAS1 = 15226.0
BIAS2 = 14724.0
CLAMP = -14724.0

NL = 3  # LR ring (lookahead W=2)
W = 2


def build_kernel(a, c, s, tau, h, repeat=1):
    scale = 86400.0 * float(h)
    K1G = -float(c) * 1024.0 * LOG2E * GAMMA
    K1E = -float(c) * 1024.0 * LOG2E
    K0 = -float(a) * 1024.0 * LOG2E
    nc = bacc.Bacc()

    gm_in = nc.declare_dram_parameter("gm", [P, NC, NV], BF16, isOutput=False)
    eye_in = nc.declare_dram_parameter("eye", [P, P], BF16, isOutput=False)
    out_ext = nc.declare_dram_parameter("out", [P, NC, S], F32, isOutput=True)

    with ExitStack() as ctx:
        tc = ctx.enter_context(tile.TileContext(nc))
        pool = ctx.enter_context(tc.tile_pool(name="p", bufs=1))

        GM = pool.tile([P, NC, NV], BF16)
        nc.sync.dma_start(out=GM[:], in_=gm_in[:])
        EYE = pool.tile([P, P], BF16)
        nc.sync.dma_start(out=EYE[:], in_=eye_in[:])

        NEG2 = pool.tile([P, NC, S], F16)
        LR = [pool.tile([P, 2, NC, S], F16, name=f"LR{r}") for r in range(NL)]
        QK = [pool.tile([P, 4096], F16, name=f"QK{r}") for r in range(2)]
        U1 = [pool.tile([P, 4096], I16, name=f"U1{r}") for r in range(2)]
        U2 = [pool.tile([P, 4096], I16, name=f"U2{r}") for r in range(2)]
        SCO = [pool.tile([P, 4097], F32, name=f"SC{r}") for r in range(2)]
        PP = [pool.tile([P, 32], F32, name=f"PP{r}") for r in range(2)]
        QF = [pool.tile([P, NC], F16, name=f"QF{r}") for r in range(2)]
        UF1 = [pool.tile([P, NC], I16, name=f"UF1{r}") for r in range(2)]
        UF2 = [pool.tile([P, NC], I16, name=f"UF2{r}") for r in range(2)]
        TF = [pool.tile([P, NC], F16, name=f"TF{r}") for r in range(2)]
        QF2 = [pool.tile([P, NC, 4], F16, name=f"QG{r}") for r in range(2)]
        UG1 = [pool.tile([P, NC, 4], I16, name=f"UG1{r}") for r in range(2)]
        UG2 = [pool.tile([P, NC, 4], I16, name=f"UG2{r}") for r in range(2)]
        TG = [pool.tile([P, NC, 4], F16, name=f"TG{r}") for r in range(2)]
        QF3 = [pool.tile([P, NC, 4], F16, name=f"QH{r}") for r in range(2)]
        UH1 = [pool.tile([P, NC, 4], I16, name=f"UH1{r}") for r in range(2)]
        UH2 = [pool.tile([P, NC, 4], I16, name=f"UH2{r}") for r in range(2)]
        TH = [pool.tile([P, NC, 4], F16, name=f"TH{r}") for r in range(2)]
        TF2 = [pool.tile([P, NC], F16, name=f"TF2{r}") for r in range(2)]
        BIAS = pool.tile([P, 1], F32)
        nc.vector.memset(BIAS[:], -float(tau) / float(s))

        DD = ctx.enter_context(nc.psum_tensor([P, 2, S, NC], F32))

        gma = GM[:]
        eyea = EYE[:]
        dda = DD[:]

        def mov_ap(v, ncols):
            # gm[p, bc, v] broadcast over ncols j-columns: free (j, bc)
            return bass.AP(gma.tensor, gma.offset + v,
                           [gma.ap[0], [0, ncols], [NV, NC]])

        def emit_dup(k):
            # D_odd (tile 0): prefix += g2o_k over cols [0, 2k); init col 2k = io_k
            # D_even (tile 1): prefix += g2e_k over cols [0, 2k+1); init col 2k+1 = ie_k
            for t, (pref_j, vpref, init_j, vinit) in enumerate(
                ((2 * k, 4 * k + 0, 2 * k, 4 * k + 2),
                 (2 * k + 1, 4 * k + 1, 2 * k + 1, 4 * k + 3))
            ):
                if t == 1 and 2 * k + 2 >= S:
                    continue
                base = t * (S * NC)
                nf = pref_j * NC
                c0 = 0
                while c0 < nf:
                    c1 = min(c0 + 512, nf)
                    outap = bass.AP(dda.tensor, dda.offset + base + c0,
                                    [dda.ap[0], [1, c1 - c0]])
                    nc.tensor.matmul(outap, eyea, mov_ap(vpref, (c1 - c0) // NC),
                                     start=False, stop=False, skip_group_check=True)
                    c0 = c1
                outap = bass.AP(dda.tensor, dda.offset + base + init_j * NC,
                                [dda.ap[0], [1, NC]])
                nc.tensor.matmul(outap, eyea, mov_ap(vinit, 1),
                                 start=False, stop=False, skip_group_check=True)

        def emit_ln(k):
            r = k % NL
            hi = 2 * k + 2
            R = 2 if 2 * k + 2 < S else 1
            lra = LR[r][:]
            inap = bass.AP(dda.tensor, dda.offset,
                           [dda.ap[0], [S * NC, R], [NC, hi], [1, NC]])
            outap = bass.AP(lra.tensor, lra.offset,
                            [lra.ap[0], [NC * S, R], [1, hi], [S, NC]])
            nc.scalar.activation(outap, inap, AF.Ln, scale=scale)

        for _rep in range(repeat):
            nc.vector.memset(NEG2[:], float(np.float16(K0)))
            nc.vector.memset(DD[:], 0.0)
            nc.gpsimd.memset(SCO[0][:, 0:1], 0.0)
            nc.gpsimd.memset(SCO[1][:, 0:1], 0.0)

            for k in range(W):
                emit_dup(k)
                emit_ln(k)

            for k in range(NK):
                if k + W < NK:
                    emit_dup(k + W)
                    emit_ln(k + W)
                r = k % NL
                x = k % 2
                jb = max(2 * k - 1, 0)  # bulk cols [0, 2k-1): two-step slack
                R = 2 if 2 * k + 2 < S else 1
                ng = R * NC  # scan groups
                lra = LR[r][:]
                qka = QK[x][:]
                u1a = U1[x][:]
                u2a = U2[x][:]
                sca = SCO[x][:]
                ppa = PP[x][:]
                n2a = NEG2[:]

                if jb > 0:
                    # QK = LR[:, 0:R, :, 0:jb] * NEG2[bcast] -> compact [R,NC,jb]
                    in0 = bass.AP(lra.tensor, lra.offset,
                                  [lra.ap[0], [NC * S, R], [S, NC], [1, jb]])
                    in1 = bass.AP(n2a.tensor, n2a.offset,
                                  [n2a.ap[0], [0, R], [S, NC], [1, jb]])
                    outq = bass.AP(qka.tensor, qka.offset,
                                   [qka.ap[0], [NC * jb, R], [jb, NC], [1, jb]])
                    nc.gpsimd.tensor_tensor(out=outq, in0=in0, in1=in1, op=ALU.mult)

                    # U1/U2 = (QK max CLAMP) + bias -> i16 (flat 2D, 4x)
                    qf2d = bass.AP(qka.tensor, qka.offset, [qka.ap[0], [1, ng * jb]])
                    o1 = bass.AP(u1a.tensor, u1a.offset, [u1a.ap[0], [1, ng * jb]])
                    o2 = bass.AP(u2a.tensor, u2a.offset, [u2a.ap[0], [1, ng * jb]])
                    nc.vector.tensor_scalar(out=o1, in0=qf2d, scalar1=CLAMP,
                                            scalar2=BIAS1, op0=ALU.max, op1=ALU.add)
                    nc.vector.tensor_scalar(out=o2, in0=qf2d, scalar1=CLAMP,
                                            scalar2=BIAS2, op0=ALU.max, op1=ALU.add)

                    # DVE scan: cum = cumsum(bitcast(U1) + bitcast(U2)), f32
                    scout = bass.AP(sca.tensor, sca.offset + 1,
                                    [sca.ap[0], [1, ng * jb]])
                    nc.vector.tensor_tensor_scan(
                        out=scout, data0=o1.bitcast(F16), data1=o2.bitcast(F16),
                        initial=0.0, op0=ALU.add, op1=ALU.add)

                    # ends-diff: P[g] = cum[(g+1)*jb] - cum[g*jb]
                    e0 = bass.AP(sca.tensor, sca.offset + jb, [sca.ap[0], [jb, ng]])
                    e1 = bass.AP(sca.tensor, sca.offset, [sca.ap[0], [jb, ng]])
                    op = bass.AP(ppa.tensor, ppa.offset, [ppa.ap[0], [1, ng]])
                    nc.gpsimd.tensor_tensor(out=op, in0=e0, in1=e1, op=ALU.subtract)

                    # NEG2[:, :, a:a+R] = P * K1G + K0 (fresh fixed up below)
                    inp = bass.AP(ppa.tensor, ppa.offset, [ppa.ap[0], [NC, R], [1, NC]])
                    onp = bass.AP(n2a.tensor, n2a.offset + 2 * k + 1,
                                  [n2a.ap[0], [1, R], [S, NC]])
                    nc.gpsimd.tensor_scalar(out=onp, in0=inp, scalar1=K1G,
                                            scalar2=K0, op0=ALU.mult, op1=ALU.add)

                # mini A: fresh j in [jb, 2k+1) for row a -> fix col 2k+1
                na = 2 * k + 1 - jb  # 1 (k=0) or 2
                qfa = QF2[x][:]
                i0 = bass.AP(lra.tensor, lra.offset + jb,
                             [lra.ap[0], [S, NC], [1, na]])
                i1 = bass.AP(n2a.tensor, n2a.offset + jb,
                             [n2a.ap[0], [S, NC], [1, na]])
                nc.gpsimd.tensor_tensor(
                    out=bass.AP(qfa.tensor, qfa.offset,
                                [qfa.ap[0], [4, NC], [1, na]]),
                    in0=i0, in1=i1, op=ALU.mult)
                ua = bass.AP(qfa.tensor, qfa.offset, [qfa.ap[0], [4, NC], [1, na]])
                nc.gpsimd.tensor_scalar(
                    out=bass.AP(UG1[x][:].tensor, UG1[x][:].offset,
                                [UG1[x][:].ap[0], [4, NC], [1, na]]),
                    in0=ua, scalar1=CLAMP, scalar2=BIAS1, op0=ALU.max, op1=ALU.add)
                nc.gpsimd.tensor_scalar(
                    out=bass.AP(UG2[x][:].tensor, UG2[x][:].offset,
                                [UG2[x][:].ap[0], [4, NC], [1, na]]),
                    in0=ua, scalar1=CLAMP, scalar2=BIAS2, op0=ALU.max, op1=ALU.add)
                tb1 = UG1[x][:].bitcast(F16)
                tb2 = UG2[x][:].bitcast(F16)
                if na == 1:
                    nc.gpsimd.tensor_tensor(
                        out=TF[x][:],
                        in0=bass.AP(tb1.tensor, tb1.offset, [tb1.ap[0], [4, NC]]),
                        in1=bass.AP(tb2.tensor, tb2.offset, [tb2.ap[0], [4, NC]]),
                        op=ALU.add)
                else:
                    # TF = (T1[0]+T2[0]) + (T1[1]+T2[1]) via two adds
                    nc.gpsimd.tensor_tensor(
                        out=bass.AP(TG[x][:].tensor, TG[x][:].offset,
                                    [TG[x][:].ap[0], [2, NC], [1, 2]]),
                        in0=bass.AP(tb1.tensor, tb1.offset,
                                    [tb1.ap[0], [4, NC], [1, 2]]),
                        in1=bass.AP(tb2.tensor, tb2.offset,
                                    [tb2.ap[0], [4, NC], [1, 2]]),
                        op=ALU.add)
                    tga = TG[x][:]
                    nc.gpsimd.tensor_tensor(
                        out=TF[x][:],
                        in0=bass.AP(tga.tensor, tga.offset, [tga.ap[0], [2, NC]]),
                        in1=bass.AP(tga.tensor, tga.offset + 1, [tga.ap[0], [2, NC]]),
                        op=ALU.add)
                oa = bass.AP(n2a.tensor, n2a.offset + 2 * k + 1,
                             [n2a.ap[0], [S, NC]])
                nc.gpsimd.tensor_scalar(out=TF[x][:], in0=TF[x][:], scalar1=K1G,
                                        scalar2=0.0, op0=ALU.mult, op1=ALU.add)
                nc.gpsimd.tensor_tensor(out=oa, in0=TF[x][:], in1=oa, op=ALU.add)

                if R == 2:
                    # mini B: fresh j in [jb, 2k+2) for row b -> fix col 2k+2
                    nb = 2 * k + 2 - jb  # 2 (k=0) or 3
                    qfb = QF3[x][:]
                    i0 = bass.AP(lra.tensor, lra.offset + NC * S + jb,
                                 [lra.ap[0], [S, NC], [1, nb]])
                    i1 = bass.AP(n2a.tensor, n2a.offset + jb,
                                 [n2a.ap[0], [S, NC], [1, nb]])
                    nc.gpsimd.tensor_tensor(
                        out=bass.AP(qfb.tensor, qfb.offset,
                                    [qfb.ap[0], [4, NC], [1, nb]]),
                        in0=i0, in1=i1, op=ALU.mult)
                    ub = bass.AP(qfb.tensor, qfb.offset, [qfb.ap[0], [4, NC], [1, nb]])
                    nc.gpsimd.tensor_scalar(
                        out=bass.AP(UH1[x][:].tensor, UH1[x][:].offset,
                                    [UH1[x][:].ap[0], [4, NC], [1, nb]]),
                        in0=ub, scalar1=CLAMP, scalar2=BIAS1,
                        op0=ALU.max, op1=ALU.add)
                    nc.gpsimd.tensor_scalar(
                        out=bass.AP(UH2[x][:].tensor, UH2[x][:].offset,
                                    [UH2[x][:].ap[0], [4, NC], [1, nb]]),
                        in0=ub, scalar1=CLAMP, scalar2=BIAS2,
                        op0=ALU.max, op1=ALU.add)
                    hb1 = UH1[x][:].bitcast(F16)
                    hb2 = UH2[x][:].bitcast(F16)
                    # TH[0:nb] = T1 + T2
                    nc.gpsimd.tensor_tensor(
                        out=bass.AP(TH[x][:].tensor, TH[x][:].offset,
                                    [TH[x][:].ap[0], [4, NC], [1, nb]]),
                        in0=bass.AP(hb1.tensor, hb1.offset,
                                    [hb1.ap[0], [4, NC], [1, nb]]),
                        in1=bass.AP(hb2.tensor, hb2.offset,
                                    [hb2.ap[0], [4, NC], [1, nb]]),
                        op=ALU.add)
                    tha = TH[x][:]
                    # sum nb cols: TF2 = TH[0] + TH[1] (+ TH[2])
                    nc.gpsimd.tensor_tensor(
                        out=TF2[x][:],
                        in0=bass.AP(tha.tensor, tha.offset, [tha.ap[0], [4, NC]]),
                        in1=bass.AP(tha.tensor, tha.offset + 1, [tha.ap[0], [4, NC]]),
                        op=ALU.add)
                    if nb == 3:
                        nc.gpsimd.tensor_tensor(
                            out=TF2[x][:], in0=TF2[x][:],
                            in1=bass.AP(tha.tensor, tha.offset + 2,
                                        [tha.ap[0], [4, NC]]),
                            op=ALU.add)
                    ob = bass.AP(n2a.tensor, n2a.offset + 2 * k + 2,
                                 [n2a.ap[0], [S, NC]])
                    nc.gpsimd.tensor_scalar(out=TF2[x][:], in0=TF2[x][:], scalar1=K1G,
                                            scalar2=0.0, op0=ALU.mult, op1=ALU.add)
                    nc.gpsimd.tensor_tensor(out=ob, in0=TF2[x][:], in1=ob, op=ALU.add)

        # epilogue
        SS = pool.tile([P, NC, S], F32)
        nc.vector.tensor_scalar(
            out=SS[:, :, 1:S], in0=NEG2[:, :, 1:S],
            scalar1=-K0, scalar2=1.0 / K1E,
            op0=ALU.add, op1=ALU.mult,
        )
        M = pool.tile([P, NC, S], F32)
        nc.scalar.activation(M[:, :, 1:S], SS[:, :, 1:S], AF.Ln)
        O = pool.tile([P, NC, S], F32)
        nc.vector.memset(O[:, :, 0:1], 0.0)
        nc.scalar.activation(
            O[:, :, 1:S], M[:, :, 1:S], AF.Sigmoid,
            scale=1.0 / float(s), bias=BIAS[:],
        )
        nc.sync.dma_start(out=out_ext[:], in_=O[:])

    nc.compile()
    return nc


def make_in_maps(sp: np.ndarray) -> list:
    bf = mybir.dt.np(BF16)
    eye = np.eye(P, dtype=np.float32).astype(bf)
    in_maps = []
    for ci in range(N_CORES):
        t = sp[:, ci * B : (ci + 1) * B].astype(np.float64)  # [S, B]
        gaps = np.empty_like(t)
        gaps[0] = t[0]
        gaps[1:] = t[1:] - t[:-1]
        vals = np.zeros((NV, B), np.float64)
        for k in range(NK):
            if k >= 1:
                vals[4 * k + 0] = t[2 * k + 1] - t[2 * k - 1]  # g2o
            if 2 * k + 2 < S:
                vals[4 * k + 1] = t[2 * k + 2] - t[2 * k]      # g2e
                vals[4 * k + 3] = gaps[2 * k + 2]              # ie
            vals[4 * k + 2] = gaps[2 * k + 1]                  # io
        # gm[p, bc, v] = vals[v, bc*128 + p]
        gm = np.ascontiguousarray(
            vals.astype(np.float32).reshape(NV, NC, P).transpose(2, 1, 0)
        ).astype(bf)
        in_maps.append({"gm": gm, "eye": eye})
    return in_maps


def kernel(sp: np.ndarray, w: np.ndarray) -> np.ndarray:
    sp = np.ascontiguousarray(sp, dtype=np.float32)
    w = np.asarray(w, dtype=np.float32)
    a, c, s, tau, h = (float(x) for x in w)

    nc = build_kernel(a, c, s, tau, h)
    in_maps = make_in_maps(sp)

    res = run_bass_kernel_spmd(nc, in_maps, core_ids=list(range(N_CORES)))
    outs = []
    for ci in range(N_CORES):
        o = res.results[ci]["out"]  # [P, NC, S]
        outs.append(o.transpose(2, 1, 0).reshape(S, B)[1:S])
    return np.concatenate(outs, axis=1).astype(np.float32)


if __name__ == "__main__":
    rng = np.random.default_rng(0)
    spt = np.cumsum(rng.uniform(0.1, 5.0, (S, B_FULL)).astype(np.float32), axis=0)
    wt = np.asarray(
        [0.176786766570677, 0.216967308403809, 0.254893976981164,
         -0.704205679427144, 0.025], dtype=np.float32)
    o = kernel(spt, wt)
    print(o.shape, o.dtype, o[:3, :3])


# revision 12
# speedup vs baseline: 3.6138x; 3.6138x over previous
"""ACT-R recurrence kernel, v2: transposed layout + PE-maintained diff state.

Layout: batch on partitions ([128 p] x [16 bc] x [128 j] free), per core
B=2048.  All TRN2 elementwise/reduce costs scale with free size only.

D state lives in PSUM [128, 16, 128] f32, maintained by the idle PE:
  step i: psum_D[:, :, 0:i] += gap_i  (identity-stationary matmul, bf16
  moving with a stride-0 broadcast AP; one accumulating matmul per bank).
After increment i, psum_D[:, :, j] = sp_i - sp_j for j < i.  Ln reads PSUM.

Per step (lag-1 split, S_i = P_i + F_i):
  Ln_i   (ACT):  L_i = Ln(scale * psum_D[0:i])            -> f16 ring
  QF_i   (DVE):  L_i[i-1] * NEG[i-1]          [128,16,1]
  ExpF_i (ACT):  F_i = Exp(QF_i)              [128,16,1]
  mul_i+1(DVE):  QB = L_{i+1}[0:i] * NEG[0:i]  f16 (2x mode)
  ExpB   (ACT):  TB = Exp(QB)                  f16
  red    (DVE):  P_{i+1} = reduce_add_X(TB)    f32
  stt_i  (DVE):  NEG[i] = -c*F_i + NEGP[i]     (fused)
  negp   (DVE):  NEGP[i+1] = -c*P_{i+1} - a
Final: S = (NEG + a)/(-c); out = Sigmoid((Ln(S) - tau)/s).
Ln and Exp share one activation table (get_activation_tables patch).
Single input: transposed bf16 gaps.  No gpsimd, no collectives.
"""

import sys

for _p in ("/opt/trn_rl_repo",):
    if _p not in sys.path:
        sys.path.insert(0, _p)

import numpy as np
from contextlib import ExitStack

import concourse.hw_specs as hw_specs
import concourse.bacc as bacc_mod
from concourse import mybir

_orig_gat = hw_specs.get_activation_tables


def _patched_gat(arch):
    tabs = _orig_gat(arch)
    out = {}
    ln_t = mybir.ActivationFunctionType.Ln
    exp_t = mybir.ActivationFunctionType.Exp
    for name, funcs in tabs.items():
        f = set(funcs)
        if name != "natural_log_exp_and_others":
            f.discard(ln_t)
            f.discard(exp_t)
        out[name] = f
    return out


bacc_mod.get_activation_tables = _patched_gat

import concourse.bass as bass
import concourse.bacc as bacc
import concourse.tile as tile
from concourse.bass_utils import run_bass_kernel_spmd

S = 128
B_FULL = 16384
N_CORES = 8
B = B_FULL // N_CORES  # 2048 per core
P = 128
NC = B // P  # 16

F32 = mybir.dt.float32
F16 = mybir.dt.float16
BF16 = mybir.dt.bfloat16
AF = mybir.ActivationFunctionType
ALU = mybir.AluOpType

NL = 5  # L ring slots (W=3 lookahead on the D/Ln pipeline)
W = 3


def build_kernel(a, c, s, tau, h, repeat=1):
    scale = 86400.0 * float(h)
    nc = bacc.Bacc()

    grt_in = nc.declare_dram_parameter("grt", [P, NC, S], BF16, isOutput=False)
    eye_in = nc.declare_dram_parameter("eye", [P, P], BF16, isOutput=False)
    out_ext = nc.declare_dram_parameter("out", [P, S, NC], F32, isOutput=True)

    with ExitStack() as ctx:
        tc = ctx.enter_context(tile.TileContext(nc))
        pool = ctx.enter_context(tc.tile_pool(name="p", bufs=1))

        GRT = pool.tile([P, NC, S], BF16)
        nc.sync.dma_start(out=GRT[:], in_=grt_in[:])
        EYE = pool.tile([P, P], BF16)
        nc.sync.dma_start(out=EYE[:], in_=eye_in[:])

        NEG = pool.tile([P, S, NC], F16)
        NEGP = pool.tile([P, S, NC], F32)

        LR = [pool.tile([P, S, NC], F16, name=f"LR{r}") for r in range(NL)]
        QB = [pool.tile([P, S, NC], F16, name=f"QB{r}") for r in range(3)]
        TB = [pool.tile([P, S, NC], F16, name=f"TB{r}") for r in range(3)]
        QF = [pool.tile([P, 1, NC], F16, name=f"QF{r}") for r in range(3)]
        TF = [pool.tile([P, 1, NC], F32, name=f"TF{r}") for r in range(3)]
        PR = [pool.tile([P, 1, NC], F32, name=f"PR{r}") for r in range(3)]
        BIAS = pool.tile([P, 1], F32)
        nc.vector.memset(BIAS[:], -float(tau) / float(s))

        psum_D = ctx.enter_context(nc.psum_tensor([P, S, NC], F32))

        def emit_inc(k):
            # psum_D[:, 0:k, :] += gap_k ; j-outer layout makes the prefix
            # contiguous: one matmul per touched 512-col psum bank.
            nj_total = k * NC
            j0 = 0
            while j0 * NC < nj_total:
                j1 = min(k, (j0 * NC + 512) // NC)
                pd = psum_D[:, j0:j1, :]
                outap = bass.AP(pd.tensor, pd.offset,
                                [pd.ap[0], [1, (j1 - j0) * NC]])
                g = GRT[:, 0, k : k + 1]
                mov = bass.AP(g.tensor, g.offset,
                              [g.ap[0], [0, j1 - j0], [S, NC]])
                nc.tensor.matmul(
                    outap, EYE[:], mov,
                    start=False, stop=False, skip_group_check=True,
                )
                j0 = j1

        def emit_ln(i, r):
            nc.scalar.activation(
                LR[r][:, 0:i, :], psum_D[:, 0:i, :], AF.Ln, scale=scale
            )

        for _rep in range(repeat):
            nc.vector.memset(NEG[:], -float(a))
            nc.vector.memset(NEGP[:], -float(a))
            nc.vector.memset(psum_D[:], 0.0)

            # prime the D/Ln pipeline W steps deep
            for k in range(1, min(W + 1, S)):
                emit_inc(k)
                emit_ln(k, k % NL)

            for i in range(1, S):
                r = i % NL
                x = i % 3
                # advance D for step i+W (waits Ln_{i+W-1}, which was
                # emitted last iteration — PE runs in ACT's shadow)
                if i + W < S:
                    emit_inc(i + W)
                # fresh chain for step i
                nc.vector.tensor_tensor(
                    out=QF[x][:], in0=LR[r][:, i - 1 : i, :],
                    in1=NEG[:, i - 1 : i, :], op=ALU.mult,
                )
                nc.scalar.activation(TF[x][:], QF[x][:], AF.Exp)
                # bulk for step i+1 over j < i
                if i + 1 < S:
                    r1 = (i + 1) % NL
                    x1 = (i + 1) % 3
                    nc.vector.tensor_tensor(
                        out=QB[x1][:, 0:i, :], in0=LR[r1][:, 0:i, :],
                        in1=NEG[:, 0:i, :], op=ALU.mult,
                    )
                    nc.scalar.activation(
                        TB[x1][:, 0:i, :], QB[x1][:, 0:i, :], AF.Exp
                    )
                if i + W < S:
                    emit_ln(i + W, (i + W) % NL)
                nc.vector.scalar_tensor_tensor(
                    out=NEG[:, i : i + 1, :], in0=TF[x][:], scalar=-float(c),
                    in1=NEGP[:, i : i + 1, :], op0=ALU.mult, op1=ALU.add,
                )
                if i + 1 < S:
                    tb = TB[x1][:, 0:i, :]
                    tbt = bass.AP(tb.tensor, tb.offset,
                                  [tb.ap[0], [1, NC], [NC, i]])
                    nc.vector.tensor_reduce(
                        out=PR[x1][:], in_=tbt,
                        axis=mybir.AxisListType.X, op=ALU.add,
                        opt_input=False,
                    )
                    nc.vector.tensor_scalar(
                        out=NEGP[:, i + 1 : i + 2, :], in0=PR[x1][:],
                        scalar1=-float(c), scalar2=-float(a),
                        op0=ALU.mult, op1=ALU.add,
                    )

        # epilogue
        SS = pool.tile([P, S, NC], F32)
        nc.vector.tensor_scalar(
            out=SS[:, 1:S, :], in0=NEG[:, 1:S, :],
            scalar1=float(a), scalar2=-1.0 / float(c),
            op0=ALU.add, op1=ALU.mult,
        )
        M = pool.tile([P, S, NC], F32)
        nc.scalar.activation(M[:, 1:S, :], SS[:, 1:S, :], AF.Ln)
        O = pool.tile([P, S, NC], F32)
        nc.vector.memset(O[:, 0:1, :], 0.0)
        nc.scalar.activation(
            O[:, 1:S, :], M[:, 1:S, :], AF.Sigmoid,
            scale=1.0 / float(s), bias=BIAS[:],
        )
        nc.sync.dma_start(out=out_ext[:], in_=O[:])

    nc.compile()
    return nc


def make_in_maps(sp: np.ndarray) -> list:
    bf = mybir.dt.np(BF16)
    eye = np.eye(P, dtype=np.float32).astype(bf)
    in_maps = []
    for ci in range(N_CORES):
        shard = sp[:, ci * B : (ci + 1) * B].astype(np.float32)  # [S, B]
        gaps = np.empty_like(shard)
        gaps[0] = shard[0]
        gaps[1:] = shard[1:] - shard[:-1]
        # grt[p, bc, j] = gaps[j, bc*128 + p]
        grt = np.ascontiguousarray(
            gaps.reshape(S, NC, P).transpose(2, 1, 0)
        ).astype(bf)
        in_maps.append({"grt": grt, "eye": eye})
    return in_maps


def kernel(sp: np.ndarray, w: np.ndarray) -> np.ndarray:
    sp = np.ascontiguousarray(sp, dtype=np.float32)
    w = np.asarray(w, dtype=np.float32)
    a, c, s, tau, h = (float(x) for x in w)

    nc = build_kernel(a, c, s, tau, h)
    in_maps = make_in_maps(sp)

    res = run_bass_kernel_spmd(nc, in_maps, core_ids=list(range(N_CORES)))
    outs = []
    for ci in range(N_CORES):
        o = res.results[ci]["out"]  # [P, S, NC]
        outs.append(o.transpose(1, 2, 0).reshape(S, B)[1:S])
    return np.concatenate(outs, axis=1).astype(np.float32)


if __name__ == "__main__":
    rng = np.random.default_rng(0)
    spt = np.cumsum(rng.uniform(0.1, 5.0, (S, B_FULL)).astype(np.float32), axis=0)
    wt = np.asarray(
        [0.176786766570677, 0.216967308403809, 0.254893976981164,
         -0.704205679427144, 0.025], dtype=np.float32)
    o = kernel(spt, wt)
    print(o.shape, o.dtype, o[:3, :3])

